# revision 1
# baseline (speedup 1.0000x reference)
"""Trainium2 Bass kernel for nn_ExpandedSiameseConcat.

Reference semantics (N=1024 anchors, C=32 classes, K=32/class, F=1024):
  out row r (r = i*63 + j, anchor i, slot j in [0,63)):
    expanded[r, 0:F]    = curr[i]
    expanded[r, F:2F]   = curr[src(i, j)]
  where src(i,j) = c*K + j                      for j < K   (positives, c = targets[i])
        src(i,j) = nc*K + rand_idx[i, d]        for j >= K  (negatives, d = j-K,
                                                 nc = d + (d >= c))
  new_targets[r] = 1 if j < K else 0
  rand_idx = jax.random.randint(jax.random.key(42), (N, C-1), 1, K)

All indices are host-computable, so the device work is a pure row-gather
expand: HBM curr rows -> SBUF tiles (via GPSIMD dma_gather, token k lands in
partition k%128, slot k//128) -> contiguous HBM writes of the output.

Sharding: data-parallel over anchors. Core c handles anchors
[c*128, (c+1)*128) -> output rows [c*8064, (c+1)*8064). curr is replicated;
per-core int16 index tables are inputs; no cross-core communication.
"""

import sys

if "/opt/trn_rl_repo" not in sys.path:
    sys.path.insert(0, "/opt/trn_rl_repo")

import numpy as np

N, C, K, F = 1024, 32, 32, 1024
NCORES = 8
APC = N // NCORES            # anchors per core = 128
RPA = K + C - 1              # output rows per anchor = 63
RPC = APC * RPA              # output rows per core = 8064
Q = 7                        # gather slots per supertile
TROWS = 128 * Q              # rows per supertile = 896
NT = RPC // TROWS            # supertiles per core = 9
IDXC = TROWS // 16           # idx columns per gather = 56
NGATHER = 2 * NT             # gathers per core (anchor + src per supertile)
NBUF = 2                     # SBUF double buffering

_CACHE: dict = {}


def _build_nc():
    import concourse.bacc as bacc
    import concourse.bass as bass
    import concourse.mybir as mybir
    from concourse.library_config import mlp
    from contextlib import ExitStack

    nc = bacc.Bacc("TRN2")
    curr = nc.dram_tensor("curr", [N, F], mybir.dt.float32, kind="ExternalInput")
    idx = nc.dram_tensor(
        "idx", [128, NGATHER * IDXC], mybir.dt.int16, kind="ExternalInput"
    )
    out = nc.dram_tensor("out", [RPC, 2 * F], mybir.dt.float32, kind="ExternalOutput")

    with ExitStack() as stack, nc.Block() as block:
        idx_sb = stack.enter_context(
            nc.sbuf_tensor("idx_sb", [128, NGATHER * IDXC], mybir.dt.int16)
        )
        tiles_a = [
            stack.enter_context(nc.sbuf_tensor(f"ta{b}", [128, Q, F], mybir.dt.float32))
            for b in range(NBUF)
        ]
        tiles_b = [
            stack.enter_context(nc.sbuf_tensor(f"tb{b}", [128, Q, F], mybir.dt.float32))
            for b in range(NBUF)
        ]
        io = stack.enter_context(nc.semaphore("io"))
        gsem = [stack.enter_context(nc.semaphore(f"g{b}")) for b in range(NBUF)]
        wsem = [stack.enter_context(nc.semaphore(f"w{b}")) for b in range(NBUF)]

        @block.gpsimd
        def _(gp):
            gp.load_library(mlp)
            gp.dma_start(idx_sb[:, :], idx[:, :]).then_inc(io, 16)
            gp.wait_ge(io, 16)
            for t in range(NT):
                b = t % NBUF
                if t >= NBUF:
                    # both writes of iteration t-NBUF on this buffer done
                    gp.wait_ge(wsem[b], 32 * (t // NBUF))
                ia = idx_sb[:, (2 * t) * IDXC : (2 * t + 1) * IDXC]
                ib = idx_sb[:, (2 * t + 1) * IDXC : (2 * t + 2) * IDXC]
                gp.dma_gather(
                    tiles_a[b][:], curr[:, :], ia, TROWS, TROWS, F
                ).then_inc(gsem[b], 16)
                gp.dma_gather(
                    tiles_b[b][:], curr[:, :], ib, TROWS, TROWS, F
                ).then_inc(gsem[b], 16)

        @block.sync
        def _(sync):
            for t in range(NT):
                b = t % NBUF
                sync.wait_ge(gsem[b], 32 * (t // NBUF + 1))
                base = t * TROWS
                # row (base + p + 128*s) <- tile[p, s, :]; col halves 0 / F
                hbm_a = bass.AP(
                    out,
                    base * 2 * F,
                    [[2 * F, 128], [2 * F * 128, Q], [1, F]],
                )
                hbm_b = bass.AP(
                    out,
                    base * 2 * F + F,
                    [[2 * F, 128], [2 * F * 128, Q], [1, F]],
                )
                sync.dma_start(hbm_a, tiles_a[b][:]).then_inc(wsem[b], 16)
                sync.dma_start(hbm_b, tiles_b[b][:]).then_inc(wsem[b], 16)
            for b in range(NBUF):
                sync.wait_ge(wsem[b], 32 * ((NT - 1 - b) // NBUF + 1))

    nc.compile()
    return nc


def _rand_idx() -> np.ndarray:
    """Bit-exact reproduction of the reference's negative-sample indices."""
    import jax

    with jax.default_device(jax.devices("cpu")[0]):
        r = jax.random.randint(jax.random.key(42), (N, C - 1), 1, K)
        return np.asarray(r)


def _wrap16(tokens: np.ndarray) -> np.ndarray:
    """Token list -> [128, len/16] int16 idx layout (token i at [i%16, i//16],
    replicated across the 8 Q7-core partition stripes)."""
    arr16 = tokens.reshape(-1, 16).T.astype(np.int16)  # [16, ntok/16]
    return np.tile(arr16, (8, 1))


def _index_tables(targets: np.ndarray) -> list[np.ndarray]:
    """Per-core [128, NGATHER*IDXC] int16 gather-index tables."""
    rand_idx = _rand_idx()
    rows = np.arange(N * RPA, dtype=np.int64)
    anchors = rows // RPA
    j = rows % RPA
    c = targets.astype(np.int64)[anchors]
    d = np.clip(j - K, 0, C - 2)
    negcls = d + (d >= c)
    src = np.where(j < K, c * K + j, negcls * K + rand_idx[anchors, d])

    tables = []
    for core in range(NCORES):
        r0 = core * RPC
        blocks = []
        for t in range(NT):
            lo = r0 + t * TROWS
            hi = lo + TROWS
            blocks.append(_wrap16(anchors[lo:hi]))
            blocks.append(_wrap16(src[lo:hi]))
        tables.append(np.ascontiguousarray(np.concatenate(blocks, axis=1)))
    return tables


def kernel(curr, targets):
    from concourse.bass_utils import run_bass_kernel_spmd

    curr = np.ascontiguousarray(np.asarray(curr), dtype=np.float32)
    targets = np.asarray(targets).astype(np.int32)

    if "nc" not in _CACHE:
        _CACHE["nc"] = _build_nc()
    nc = _CACHE["nc"]

    tables = _index_tables(targets)
    in_maps = [{"curr": curr, "idx": tables[core]} for core in range(NCORES)]
    res = run_bass_kernel_spmd(nc, in_maps, core_ids=list(range(NCORES)))
    expanded = np.concatenate([r["out"] for r in res.results], axis=0)

    new_targets = np.tile(
        np.concatenate([np.ones(K, np.int32), np.zeros(C - 1, np.int32)]), N
    )
    return new_targets, expanded


# revision 6
# speedup vs baseline: 1.5910x; 1.5910x over previous
"""Trainium2 Bass kernel for nn_ExpandedSiameseConcat.

Reference semantics (N=1024 anchors, C=32 classes, K=32/class, F=1024):
  out row r (r = i*63 + j, anchor i, slot j in [0,63)):
    expanded[r, 0:F]  = curr[i]
    expanded[r, F:2F] = curr[src(i, j)]
  where src(i,j) = c*K + j               for j < K   (positives, c = targets[i])
        src(i,j) = nc*K + rand_idx[i,d]  for j >= K  (negatives, d = j-K,
                                                      nc = d + (d >= c))
  new_targets[r] = 1 if j < K else 0
  rand_idx = jax.random.randint(jax.random.key(42), (N, C-1), 1, K)

Sharding: data-parallel over anchors; core c -> anchors [c*128, (c+1)*128),
output rows [c*8064, (c+1)*8064). No cross-core communication.

Device strategy (memory-regime; writes are irreducible at ~65 MB/core, so
the optimization target is HBM *reads*):
  - one dma_gather pulls the core's 128 anchor rows into SBUF ("shard",
    partition p = anchor a0+p; 0.5 MB read)
  - anchor halves (33 MB) and positive halves (16.5 MB) are written straight
    from the shard with stride-0 (broadcast) SBUF read APs - zero HBM reads:
      anchor:   out[p*63+j,    0:F] = shard[p]            (j broadcast)
      positive: out[(cls*32+m)*63 + j, F:2F] = shard[cls*32+j]  (m broadcast)
  - negatives (random rows of other classes) use dma_gather from HBM curr
    (15.5 MB read), token k = s*128+p -> out[p*63+32+s, F:2F]
Total HBM traffic/core ~81 MB vs 132 MB for the naive all-gather version.
"""

import sys

if "/opt/trn_rl_repo" not in sys.path:
    sys.path.insert(0, "/opt/trn_rl_repo")

import numpy as np

N, C, K, F = 1024, 32, 32, 1024
NCORES = 8
APC = N // NCORES            # anchors per core = 128
RPA = K + C - 1              # output rows per anchor = 63
RPC = APC * RPA              # output rows per core = 8064
NNEG = C - 1                 # negative slots per anchor = 31
CPC = APC // K               # classes per core = 4

# negative gather chunking (slots of the j-32 dimension)
NEG_CHUNKS = [(0, 8), (8, 8), (16, 8), (24, 7)]
# anchor-half write chunking (slots of the j dimension)
ANC_CHUNKS = [(0, 16), (16, 16), (32, 16), (48, 15)]

# idx table: gather 0 = shard (128 tokens), then one gather per neg chunk
IDX_COLS = (APC + APC * NNEG) // 16  # 256

_CACHE: dict = {}


def _build_nc():
    import concourse.bacc as bacc
    import concourse.bass as bass
    import concourse.mybir as mybir
    from concourse.library_config import mlp
    from contextlib import ExitStack

    R = 2 * F  # output row length in elements

    nc = bacc.Bacc("TRN2")
    curr = nc.dram_tensor("curr", [N, F], mybir.dt.float32, kind="ExternalInput")
    idx = nc.dram_tensor("idx", [128, IDX_COLS], mybir.dt.int16, kind="ExternalInput")
    out = nc.dram_tensor("out", [RPC, R], mybir.dt.float32, kind="ExternalOutput")

    with ExitStack() as stack, nc.Block() as block:
        idx_sb = stack.enter_context(
            nc.sbuf_tensor("idx_sb", [128, IDX_COLS], mybir.dt.int16)
        )
        shard = stack.enter_context(
            nc.sbuf_tensor("shard", [128, 1, F], mybir.dt.float32)
        )
        neg = stack.enter_context(
            nc.sbuf_tensor("neg", [128, NNEG, F], mybir.dt.float32)
        )
        io = stack.enter_context(nc.semaphore("io"))
        ssem = stack.enter_context(nc.semaphore("ssem"))
        gsem = [
            stack.enter_context(nc.semaphore(f"gsem{g}"))
            for g in range(len(NEG_CHUNKS))
        ]
        wsem = stack.enter_context(nc.semaphore("wsem"))
        nsem = stack.enter_context(nc.semaphore("nsem"))

        @block.gpsimd
        def _(gp):
            gp.load_library(mlp)
            gp.dma_start(idx_sb[:, :], idx[:, :]).then_inc(io, 16)
            gp.wait_ge(io, 16)
            # gather 0: the core's own 128 anchor rows -> shard
            gp.dma_gather(
                shard[:], curr[:, :], idx_sb[:, 0:8], APC, APC, F
            ).then_inc(ssem, 16)
            col = 8
            for g, (s0, ns) in enumerate(NEG_CHUNKS):
                ntok = 128 * ns
                gp.dma_gather(
                    neg[:, s0 : s0 + ns, :],
                    curr[:, :],
                    idx_sb[:, col : col + ntok // 16],
                    ntok,
                    ntok,
                    F,
                ).then_inc(gsem[g], 16)
                col += ntok // 16

        @block.sync
        def _(sync):
            # anchor halves + positives: pure broadcast writes from shard
            sync.wait_ge(ssem, 16)
            for j0, nj in ANC_CHUNKS:
                # out[p*63 + (j0+jj), 0:F] = shard[p]
                hbm = bass.AP(out, j0 * R, [[RPA * R, 128], [R, nj], [1, F]])
                sb = bass.AP(shard, 0, [[F, 128], [0, nj], [1, F]])
                sync.dma_start(hbm, sb).then_inc(wsem, 16)
            for cls in range(CPC):
                # out[(cls*32+m)*63 + j, F:2F] = shard[cls*32 + j]
                hbm = bass.AP(
                    out,
                    (cls * K * RPA) * R + F,
                    [[R, K], [RPA * R, K], [1, F]],
                )
                sb = bass.AP(shard, cls * K * F, [[F, K], [0, K], [1, F]])
                sync.dma_start(hbm, sb).then_inc(wsem, 16)
            sync.wait_ge(wsem, 16 * (len(ANC_CHUNKS) + CPC))

        @block.scalar
        def _(sc):
            # negatives: write gathered chunks as they land
            for g, (s0, ns) in enumerate(NEG_CHUNKS):
                sc.wait_ge(gsem[g], 16)
                hbm = bass.AP(
                    out,
                    (K + s0) * R + F,
                    [[RPA * R, 128], [R, ns], [1, F]],
                )
                sc.dma_start(hbm, neg[:, s0 : s0 + ns, :]).then_inc(nsem, 16)
            sc.wait_ge(nsem, 16 * len(NEG_CHUNKS))

    nc.compile()
    return nc


def _rand_idx() -> np.ndarray:
    """Bit-exact reproduction of the reference's negative-sample indices."""
    import jax

    with jax.default_device(jax.devices("cpu")[0]):
        r = jax.random.randint(jax.random.key(42), (N, C - 1), 1, K)
        return np.asarray(r)


def _wrap16(tokens: np.ndarray) -> np.ndarray:
    """Token list -> [128, len/16] int16 idx layout (token i at [i%16, i//16],
    replicated across the 8 Q7-core partition stripes)."""
    arr16 = tokens.reshape(-1, 16).T.astype(np.int16)
    return np.tile(arr16, (8, 1))


def _index_tables(targets: np.ndarray) -> list[np.ndarray]:
    """Per-core [128, IDX_COLS] int16 gather-index tables."""
    rand_idx = _rand_idx()
    tgt = targets.astype(np.int64)

    tables = []
    for core in range(NCORES):
        a0 = core * APC
        anchors = np.arange(a0, a0 + APC, dtype=np.int64)
        blocks = [_wrap16(anchors)]
        c = tgt[anchors]
        for s0, ns in NEG_CHUNKS:
            # token k = s_rel*128 + p  -> anchor a0+p, d = s0+s_rel
            d = (s0 + np.arange(ns, dtype=np.int64))[:, None]      # [ns, 1]
            negcls = d + (d >= c[None, :])                          # [ns, 128]
            src = negcls * K + rand_idx[anchors[None, :], d]        # [ns, 128]
            blocks.append(_wrap16(src.ravel()))
        tables.append(np.ascontiguousarray(np.concatenate(blocks, axis=1)))
    return tables


def kernel(curr, targets):
    from concourse.bass_utils import run_bass_kernel_spmd

    curr = np.ascontiguousarray(np.asarray(curr), dtype=np.float32)
    targets = np.asarray(targets).astype(np.int32)

    if "nc" not in _CACHE:
        _CACHE["nc"] = _build_nc()
    nc = _CACHE["nc"]

    tables = _index_tables(targets)
    in_maps = [{"curr": curr, "idx": tables[core]} for core in range(NCORES)]
    res = run_bass_kernel_spmd(nc, in_maps, core_ids=list(range(NCORES)))
    expanded = np.concatenate([r["out"] for r in res.results], axis=0)

    new_targets = np.tile(
        np.concatenate([np.ones(K, np.int32), np.zeros(C - 1, np.int32)]), N
    )
    return new_targets, expanded


# revision 12
# speedup vs baseline: 1.8542x; 1.1654x over previous
"""Trainium2 Bass kernel for nn_ExpandedSiameseConcat.

Reference semantics (N=1024 anchors, C=32 classes, K=32/class, F=1024):
  out row r (r = i*63 + j, anchor i, slot j in [0,63)):
    expanded[r, 0:F]  = curr[i]
    expanded[r, F:2F] = curr[src(i, j)]
  where src(i,j) = c*K + j               for j < K   (positives, c = targets[i])
        src(i,j) = nc*K + rand_idx[i,d]  for j >= K  (negatives, d = j-K,
                                                      nc = d + (d >= c))
  new_targets[r] = 1 if j < K else 0
  rand_idx = jax.random.randint(jax.random.key(42), (N, C-1), 1, K)

Sharding: data-parallel over anchors; core c -> anchors [c*128, (c+1)*128),
output rows [c*8064, (c+1)*8064). No cross-core communication.

Device strategy (memory-regime; the ~66 MB/core of output writes are
irreducible, so the optimization target is HBM *reads*):
  - one plain DMA pulls the core's 128 anchor rows into SBUF ("shard",
    partition p = anchor a0+p) at HBM offset partition_id*128 rows (0.5 MB)
  - anchor halves (33 MB) and positive halves (16.5 MB) are written straight
    from the shard with stride-0 (broadcast) SBUF read APs - zero HBM reads:
      anchor:   out[p*63+j,    0:F] = shard[p]                  (j broadcast)
      positive: out[(cls*32+m)*63 + j, F:2F] = shard[cls*32+j]  (m broadcast)
  - negatives: all of curr stays resident in SBUF (4 MB, row g at partition
    g%128, slot g//128); rows are *scatter-written* to their sampled output
    rows with GPSIMD indirect_dma_start (bypass - no destination read, one
    int32 index per partition per call, OOB sentinels skipped via bounds
    check). Round m of slot q writes the m-th consumer of each row q*128+p.
    Rounds are capped at MCAP (fill rate decays with multiplicity); the
    remaining high-multiplicity consumers (~4% of rows) go through one small
    dma_gather (HBM -> SBUF, compacted in token order) followed by
    indirect scatters from the compact tile.
Total HBM traffic/core ~71 MB (66 write + ~5 read) vs 132 MB naive.
"""

import sys

if "/opt/trn_rl_repo" not in sys.path:
    sys.path.insert(0, "/opt/trn_rl_repo")

import numpy as np

N, C, K, F = 1024, 32, 32, 1024
NCORES = 8
APC = N // NCORES            # anchors per core = 128
RPA = K + C - 1              # output rows per anchor = 63
RPC = APC * RPA              # output rows per core = 8064
NNEG = C - 1                 # negative slots per anchor = 31
CPC = APC // K               # classes per core = 4
NSLOT = N // 128             # curr rows per SBUF partition = 8
SENTINEL = 2 * RPC           # OOB scatter index (skipped by bounds check)
MCAP = 6                     # resident scatter rounds per slot

DEFAULT_CFG = dict(
    anc_chunks=(63,),          # slot counts of the anchor j dimension
    pos_engine="scalar",       # "sync" | "scalar"
    cost_twin=False,           # replace indirect scatters with equivalent-
                               # traffic plain writes (for TimelineSim only)
)

_CACHE: dict = {}


def _chunks(counts):
    out, s0 = [], 0
    for n in counts:
        out.append((s0, n))
        s0 += n
    return out


def _build_nc(lmax: int, cfg=DEFAULT_CFG):
    import concourse.bacc as bacc
    import concourse.bass as bass
    import concourse.mybir as mybir
    from concourse.library_config import mlp
    from contextlib import ExitStack

    R = 2 * F                      # output row length in elements
    nrem = lmax // 128             # remainder scatter calls
    nsc = NSLOT * MCAP + nrem      # scidx columns / total scatter calls
    anc_chunks = _chunks(cfg["anc_chunks"])
    assert sum(cfg["anc_chunks"]) == RPA and lmax % 128 == 0

    nc = bacc.Bacc("TRN2", detect_race_conditions=False)
    curr = nc.dram_tensor("curr", [N, F], mybir.dt.float32, kind="ExternalInput")
    scidx = nc.dram_tensor("scidx", [128, nsc], mybir.dt.int32, kind="ExternalInput")
    gidx = nc.dram_tensor(
        "gidx", [128, max(lmax // 16, 8)], mybir.dt.int16, kind="ExternalInput"
    )
    out = nc.dram_tensor("out", [RPC, R], mybir.dt.float32, kind="ExternalOutput")

    with ExitStack() as stack, nc.Block() as block:
        scidx_sb = stack.enter_context(
            nc.sbuf_tensor("scidx_sb", [128, nsc], mybir.dt.int32)
        )
        gidx_sb = stack.enter_context(
            nc.sbuf_tensor("gidx_sb", [128, max(lmax // 16, 8)], mybir.dt.int16)
        )
        shard = stack.enter_context(nc.sbuf_tensor("shard", [128, F], mybir.dt.float32))
        curr_sb = stack.enter_context(
            nc.sbuf_tensor("curr_sb", [128, NSLOT, F], mybir.dt.float32)
        )
        rem = stack.enter_context(
            nc.sbuf_tensor("rem", [128, max(nrem, 1), F], mybir.dt.float32)
        )
        csem = stack.enter_context(nc.semaphore("csem"))
        isem = stack.enter_context(nc.semaphore("isem"))
        gsem = stack.enter_context(nc.semaphore("gsem"))
        ssem = stack.enter_context(nc.semaphore("ssem"))
        wsem = stack.enter_context(nc.semaphore("wsem"))
        psem = stack.enter_context(nc.semaphore("psem"))
        nsem = stack.enter_context(nc.semaphore("nsem"))

        out_half_rows = bass.AP(out, 0, [[F, 2 * RPC], [1, F]])

        def scatter(gp, src_tensor, src_off, src_pstride, col):
            if cfg["cost_twin"]:
                # cost-model stand-in: the model can't price indirect DMAs
                # (it reads the full out AP size), so emit a plain write of
                # this call's actual valid-token count (per-core fill).
                fills = cfg.get("twin_fills")
                vp = max(int(fills[col]), 1) if fills is not None else 128
                hbm = bass.AP(
                    out, (K + col % NNEG) * R + F, [[RPA * R, vp], [1, F]]
                )
                sb = bass.AP(src_tensor, src_off, [[src_pstride, vp], [1, F]])
                return gp.dma_start(hbm, sb)
            return gp.indirect_dma_start(
                out=out_half_rows,
                out_offset=bass.IndirectOffsetOnAxis(
                    ap=scidx_sb[:, col : col + 1], axis=0
                ),
                in_=bass.AP(src_tensor, src_off, [[src_pstride, 128], [1, F]]),
                in_offset=None,
                bounds_check=2 * RPC - 1,
                oob_is_err=False,
            )

        @block.gpsimd
        def _(gp):
            gp.load_library(mlp)
            # curr resident: curr_sb[p, q, :] = curr[q*128 + p]
            gp.dma_start(
                curr_sb[:, :, :],
                bass.AP(curr, 0, [[F, 128], [128 * F, NSLOT], [1, F]]),
            ).then_inc(csem, 16)
            gp.dma_start(scidx_sb[:, :], scidx[:, :]).then_inc(isem, 16)
            gp.dma_start(gidx_sb[:, :], gidx[:, :]).then_inc(isem, 16)
            gp.wait_ge(isem, 32)
            # remainder gather can start as soon as the index tables are in
            if nrem:
                gp.dma_gather(
                    rem[:, :, :], curr[:, :], gidx_sb[:, : lmax // 16],
                    lmax, lmax, F,
                ).then_inc(gsem, 16)
            gp.wait_ge(csem, 16)
            for m in range(MCAP):
                for q in range(NSLOT):
                    scatter(
                        gp, curr_sb, q * F, NSLOT * F, m * NSLOT + q
                    ).then_inc(nsem, 16)
            if nrem:
                gp.wait_ge(gsem, 16)
                for l in range(nrem):
                    scatter(
                        gp, rem, l * F, max(nrem, 1) * F, NSLOT * MCAP + l
                    ).then_inc(nsem, 16)
            gp.wait_ge(nsem, 16 * nsc)

        @block.sync
        def _(sync):
            with sync.register("shard_off") as off:
                pid = sync.partition_id()
                sync.reg_mul(off, pid, APC * F)
                sync.dma_start(
                    shard[:, :], bass.AP(curr, off, [[F, 128], [1, F]])
                ).then_inc(ssem, 16)
            sync.wait_ge(ssem, 16)
            nw = 0
            for j0, nj in anc_chunks:
                # out[p*63 + (j0+jj), 0:F] = shard[p]
                hbm = bass.AP(out, j0 * R, [[RPA * R, 128], [R, nj], [1, F]])
                sb = bass.AP(shard, 0, [[F, 128], [0, nj], [1, F]])
                sync.dma_start(hbm, sb).then_inc(wsem, 16)
                nw += 1
            if cfg["pos_engine"] == "sync":
                nw += _write_pos(bass, sync, shard, out, wsem)
            sync.wait_ge(wsem, 16 * nw)

        @block.scalar
        def _(sc):
            if cfg["pos_engine"] == "scalar":
                sc.wait_ge(ssem, 16)
                npos = _write_pos(bass, sc, shard, out, psem)
                sc.wait_ge(psem, 16 * npos)

    nc.compile()
    return nc


def _write_pos(bass, eng, shard, out, sem):
    """out[(cls*32+m)*63 + j, F:2F] = shard[cls*32 + j] for j,m in [0,32)^2."""
    R = 2 * F
    for cls in range(CPC):
        hbm = bass.AP(out, (cls * K * RPA) * R + F, [[R, K], [RPA * R, K], [1, F]])
        sb = bass.AP(shard, cls * K * F, [[F, K], [0, K], [1, F]])
        eng.dma_start(hbm, sb).then_inc(sem, 16)
    return CPC


def _rand_idx() -> np.ndarray:
    """Bit-exact reproduction of the reference's negative-sample indices."""
    import jax

    with jax.default_device(jax.devices("cpu")[0]):
        r = jax.random.randint(jax.random.key(42), (N, C - 1), 1, K)
        return np.asarray(r)


def _wrap16(tokens: np.ndarray) -> np.ndarray:
    """Token list -> [128, len/16] int16 idx layout for dma_gather (token i at
    [i%16, i//16], replicated across the 8 Q7-core partition stripes)."""
    arr16 = tokens.reshape(-1, 16).T.astype(np.int16)
    return np.tile(arr16, (8, 1))


def _scatter_tables(targets: np.ndarray):
    """Per-core (scidx [128, nsc] int32, gather tokens list) + lmax.

    Resident rounds: scidx[p, m*8+q] = 2*dest+1 of the m-th consumer of curr
    row q*128+p (m < MCAP), else SENTINEL. Consumers beyond MCAP become
    remainder tokens: dma_gather pulls curr[g] into token slot t, scatter call
    t//128 sends it to scidx[t%128, 8*MCAP + t//128] = 2*dest+1.
    """
    rand_idx = _rand_idx()
    tgt = targets.astype(np.int64)

    per_core = []
    lmax = 0
    for core in range(NCORES):
        a0 = core * APC
        c = tgt[a0 : a0 + APC]                                  # [128]
        d = np.arange(NNEG, dtype=np.int64)[:, None]            # [31, 1]
        negcls = d + (d >= c[None, :])                          # [31, 128]
        g = negcls * K + rand_idx[a0 : a0 + APC, :].T           # [31, 128]
        dest = (np.arange(APC) * RPA + K)[None, :] + d          # [31, 128]
        lists: dict[int, list[int]] = {}
        for dd in range(NNEG):
            for p in range(APC):
                lists.setdefault(int(g[dd, p]), []).append(int(dest[dd, p]))
        rem_pairs = []  # (source row, dest row) beyond MCAP
        for gg, dests in lists.items():
            for dd in dests[MCAP:]:
                rem_pairs.append((gg, dd))
        lmax = max(lmax, len(rem_pairs))
        per_core.append((lists, rem_pairs))

    lmax = max(((lmax + 127) // 128) * 128, 128)
    nrem = lmax // 128
    nsc = NSLOT * MCAP + nrem

    tables = []
    for lists, rem_pairs in per_core:
        scidx = np.full((128, nsc), SENTINEL, dtype=np.int32)
        for gg, dests in lists.items():
            q, p = divmod(gg, 128)
            for m, dd in enumerate(dests[:MCAP]):
                scidx[p, m * NSLOT + q] = 2 * dd + 1
        gtok = np.zeros(lmax, dtype=np.int64)
        for t, (gg, dd) in enumerate(rem_pairs):
            gtok[t] = gg
            scidx[t % 128, NSLOT * MCAP + t // 128] = 2 * dd + 1
        tables.append((scidx, _wrap16(gtok)))
    return tables, lmax


def kernel(curr, targets):
    from concourse.bass_utils import run_bass_kernel_spmd

    curr = np.ascontiguousarray(np.asarray(curr), dtype=np.float32)
    targets = np.asarray(targets).astype(np.int32)

    tables, lmax = _scatter_tables(targets)
    key = ("nc", lmax)
    if key not in _CACHE:
        _CACHE[key] = _build_nc(lmax)
    nc = _CACHE[key]
    _CACHE["last"] = nc
    _CACHE["last_lmax"] = lmax

    in_maps = [
        {"curr": curr, "scidx": tables[core][0], "gidx": tables[core][1]}
        for core in range(NCORES)
    ]
    res = run_bass_kernel_spmd(nc, in_maps, core_ids=list(range(NCORES)))
    expanded = np.concatenate([r["out"] for r in res.results], axis=0)

    new_targets = np.tile(
        np.concatenate([np.ones(K, np.int32), np.zeros(C - 1, np.int32)]), N
    )
    return new_targets, expanded


# revision 14
# speedup vs baseline: 1.8671x; 1.0070x over previous
"""Trainium2 Bass kernel for nn_ExpandedSiameseConcat.

Reference semantics (N=1024 anchors, C=32 classes, K=32/class, F=1024):
  out row r (r = i*63 + j, anchor i, slot j in [0,63)):
    expanded[r, 0:F]  = curr[i]
    expanded[r, F:2F] = curr[src(i, j)]
  where src(i,j) = c*K + j               for j < K   (positives, c = targets[i])
        src(i,j) = nc*K + rand_idx[i,d]  for j >= K  (negatives, d = j-K,
                                                      nc = d + (d >= c))
  new_targets[r] = 1 if j < K else 0
  rand_idx = jax.random.randint(jax.random.key(42), (N, C-1), 1, K)

Sharding: data-parallel over anchors; core c -> anchors [c*128, (c+1)*128),
output rows [c*8064, (c+1)*8064). No cross-core communication.

Device strategy (memory-regime; the ~66 MB/core of output writes are
irreducible, so the optimization target is HBM *reads*):
  - one plain DMA pulls the core's 128 anchor rows into SBUF ("shard",
    partition p = anchor a0+p) at HBM offset partition_id*128 rows (0.5 MB)
  - anchor halves (33 MB) and positive halves (16.5 MB) are written straight
    from the shard with stride-0 (broadcast) SBUF read APs - zero HBM reads:
      anchor:   out[p*63+j,    0:F] = shard[p]                  (j broadcast)
      positive: out[(cls*32+m)*63 + j, F:2F] = shard[cls*32+j]  (m broadcast)
  - negatives: all of curr stays resident in SBUF (4 MB, row g at partition
    g%128, slot g//128); rows are *scatter-written* to their sampled output
    rows with GPSIMD indirect_dma_start (bypass - no destination read, one
    int32 index per partition per call, OOB sentinels skipped via bounds
    check). Round m of slot q writes the m-th consumer of each row q*128+p.
    Rounds are capped at MCAP (fill rate decays with multiplicity); the
    remaining high-multiplicity consumers (~4% of rows) go through one small
    dma_gather (HBM -> SBUF, compacted in token order) followed by
    indirect scatters from the compact tile.
Total HBM traffic/core ~71 MB (66 write + ~5 read) vs 132 MB naive.
"""

import sys

if "/opt/trn_rl_repo" not in sys.path:
    sys.path.insert(0, "/opt/trn_rl_repo")

import numpy as np

N, C, K, F = 1024, 32, 32, 1024
NCORES = 8
APC = N // NCORES            # anchors per core = 128
RPA = K + C - 1              # output rows per anchor = 63
RPC = APC * RPA              # output rows per core = 8064
NNEG = C - 1                 # negative slots per anchor = 31
CPC = APC // K               # classes per core = 4
NSLOT = N // 128             # curr rows per SBUF partition = 8
SENTINEL = 2 * RPC           # OOB scatter index (skipped by bounds check)
MCAP = 8                     # resident scatter rounds per slot

DEFAULT_CFG = dict(
    anc_chunks=(63,),          # slot counts of the anchor j dimension
    pos_engine="scalar",       # "sync" | "scalar"
    cost_twin=False,           # replace indirect scatters with equivalent-
                               # traffic plain writes (for TimelineSim only)
)

_CACHE: dict = {}


def _chunks(counts):
    out, s0 = [], 0
    for n in counts:
        out.append((s0, n))
        s0 += n
    return out


def _build_nc(lmax: int, cfg=DEFAULT_CFG):
    import concourse.bacc as bacc
    import concourse.bass as bass
    import concourse.mybir as mybir
    from concourse.library_config import mlp
    from contextlib import ExitStack

    R = 2 * F                      # output row length in elements
    nrem = lmax // 128             # remainder scatter calls
    nsc = NSLOT * MCAP + nrem      # scidx columns / total scatter calls
    anc_chunks = _chunks(cfg["anc_chunks"])
    assert sum(cfg["anc_chunks"]) == RPA and lmax % 128 == 0

    nc = bacc.Bacc("TRN2", detect_race_conditions=False)
    curr = nc.dram_tensor("curr", [N, F], mybir.dt.float32, kind="ExternalInput")
    scidx = nc.dram_tensor("scidx", [128, nsc], mybir.dt.int32, kind="ExternalInput")
    gidx = nc.dram_tensor(
        "gidx", [128, max(lmax // 16, 8)], mybir.dt.int16, kind="ExternalInput"
    )
    out = nc.dram_tensor("out", [RPC, R], mybir.dt.float32, kind="ExternalOutput")

    with ExitStack() as stack, nc.Block() as block:
        scidx_sb = stack.enter_context(
            nc.sbuf_tensor("scidx_sb", [128, nsc], mybir.dt.int32)
        )
        gidx_sb = stack.enter_context(
            nc.sbuf_tensor("gidx_sb", [128, max(lmax // 16, 8)], mybir.dt.int16)
        )
        shard = stack.enter_context(nc.sbuf_tensor("shard", [128, F], mybir.dt.float32))
        curr_sb = stack.enter_context(
            nc.sbuf_tensor("curr_sb", [128, NSLOT, F], mybir.dt.float32)
        )
        rem = stack.enter_context(
            nc.sbuf_tensor("rem", [128, max(nrem, 1), F], mybir.dt.float32)
        )
        csem = stack.enter_context(nc.semaphore("csem"))
        isem = stack.enter_context(nc.semaphore("isem"))
        gsem = stack.enter_context(nc.semaphore("gsem"))
        ssem = stack.enter_context(nc.semaphore("ssem"))
        wsem = stack.enter_context(nc.semaphore("wsem"))
        psem = stack.enter_context(nc.semaphore("psem"))
        nsem = stack.enter_context(nc.semaphore("nsem"))

        out_half_rows = bass.AP(out, 0, [[F, 2 * RPC], [1, F]])

        def scatter(gp, src_tensor, src_off, src_pstride, col):
            if cfg["cost_twin"]:
                # cost-model stand-in: the model can't price indirect DMAs
                # (it reads the full out AP size), so emit a plain write of
                # this call's actual valid-token count (per-core fill).
                fills = cfg.get("twin_fills")
                vp = max(int(fills[col]), 1) if fills is not None else 128
                hbm = bass.AP(
                    out, (K + col % NNEG) * R + F, [[RPA * R, vp], [1, F]]
                )
                sb = bass.AP(src_tensor, src_off, [[src_pstride, vp], [1, F]])
                return gp.dma_start(hbm, sb)
            return gp.indirect_dma_start(
                out=out_half_rows,
                out_offset=bass.IndirectOffsetOnAxis(
                    ap=scidx_sb[:, col : col + 1], axis=0
                ),
                in_=bass.AP(src_tensor, src_off, [[src_pstride, 128], [1, F]]),
                in_offset=None,
                bounds_check=2 * RPC - 1,
                oob_is_err=False,
            )

        @block.gpsimd
        def _(gp):
            gp.load_library(mlp)
            # curr resident: curr_sb[p, q, :] = curr[q*128 + p]
            gp.dma_start(
                curr_sb[:, :, :],
                bass.AP(curr, 0, [[F, 128], [128 * F, NSLOT], [1, F]]),
            ).then_inc(csem, 16)
            gp.dma_start(scidx_sb[:, :], scidx[:, :]).then_inc(isem, 16)
            gp.dma_start(gidx_sb[:, :], gidx[:, :]).then_inc(isem, 16)
            gp.wait_ge(isem, 32)
            # remainder gather can start as soon as the index tables are in
            if nrem:
                gp.dma_gather(
                    rem[:, :, :], curr[:, :], gidx_sb[:, : lmax // 16],
                    lmax, lmax, F,
                ).then_inc(gsem, 16)
            gp.wait_ge(csem, 16)
            for m in range(MCAP):
                for q in range(NSLOT):
                    scatter(
                        gp, curr_sb, q * F, NSLOT * F, m * NSLOT + q
                    ).then_inc(nsem, 16)
            if nrem:
                gp.wait_ge(gsem, 16)
                for l in range(nrem):
                    scatter(
                        gp, rem, l * F, max(nrem, 1) * F, NSLOT * MCAP + l
                    ).then_inc(nsem, 16)
            gp.wait_ge(nsem, 16 * nsc)

        @block.sync
        def _(sync):
            with sync.register("shard_off") as off:
                pid = sync.partition_id()
                sync.reg_mul(off, pid, APC * F)
                sync.dma_start(
                    shard[:, :], bass.AP(curr, off, [[F, 128], [1, F]])
                ).then_inc(ssem, 16)
            sync.wait_ge(ssem, 16)
            nw = 0
            for j0, nj in anc_chunks:
                # out[p*63 + (j0+jj), 0:F] = shard[p]
                hbm = bass.AP(out, j0 * R, [[RPA * R, 128], [R, nj], [1, F]])
                sb = bass.AP(shard, 0, [[F, 128], [0, nj], [1, F]])
                sync.dma_start(hbm, sb).then_inc(wsem, 16)
                nw += 1
            if cfg["pos_engine"] == "sync":
                nw += _write_pos(bass, sync, shard, out, wsem)
            sync.wait_ge(wsem, 16 * nw)

        @block.scalar
        def _(sc):
            if cfg["pos_engine"] == "scalar":
                sc.wait_ge(ssem, 16)
                npos = _write_pos(bass, sc, shard, out, psem)
                sc.wait_ge(psem, 16 * npos)

    nc.compile()
    return nc


def _write_pos(bass, eng, shard, out, sem):
    """out[(cls*32+m)*63 + j, F:2F] = shard[cls*32 + j] for j,m in [0,32)^2."""
    R = 2 * F
    for cls in range(CPC):
        hbm = bass.AP(out, (cls * K * RPA) * R + F, [[R, K], [RPA * R, K], [1, F]])
        sb = bass.AP(shard, cls * K * F, [[F, K], [0, K], [1, F]])
        eng.dma_start(hbm, sb).then_inc(sem, 16)
    return CPC


def _rand_idx() -> np.ndarray:
    """Bit-exact reproduction of the reference's negative-sample indices."""
    import jax

    with jax.default_device(jax.devices("cpu")[0]):
        r = jax.random.randint(jax.random.key(42), (N, C - 1), 1, K)
        return np.asarray(r)


def _wrap16(tokens: np.ndarray) -> np.ndarray:
    """Token list -> [128, len/16] int16 idx layout for dma_gather (token i at
    [i%16, i//16], replicated across the 8 Q7-core partition stripes)."""
    arr16 = tokens.reshape(-1, 16).T.astype(np.int16)
    return np.tile(arr16, (8, 1))


def _scatter_tables(targets: np.ndarray):
    """Per-core (scidx [128, nsc] int32, gather tokens list) + lmax.

    Resident rounds: scidx[p, m*8+q] = 2*dest+1 of the m-th consumer of curr
    row q*128+p (m < MCAP), else SENTINEL. Consumers beyond MCAP become
    remainder tokens: dma_gather pulls curr[g] into token slot t, scatter call
    t//128 sends it to scidx[t%128, 8*MCAP + t//128] = 2*dest+1.
    """
    rand_idx = _rand_idx()
    tgt = targets.astype(np.int64)

    per_core = []
    lmax = 0
    for core in range(NCORES):
        a0 = core * APC
        c = tgt[a0 : a0 + APC]                                  # [128]
        d = np.arange(NNEG, dtype=np.int64)[:, None]            # [31, 1]
        negcls = d + (d >= c[None, :])                          # [31, 128]
        g = negcls * K + rand_idx[a0 : a0 + APC, :].T           # [31, 128]
        dest = (np.arange(APC) * RPA + K)[None, :] + d          # [31, 128]
        lists: dict[int, list[int]] = {}
        for dd in range(NNEG):
            for p in range(APC):
                lists.setdefault(int(g[dd, p]), []).append(int(dest[dd, p]))
        rem_pairs = []  # (source row, dest row) beyond MCAP
        for gg, dests in lists.items():
            for dd in dests[MCAP:]:
                rem_pairs.append((gg, dd))
        lmax = max(lmax, len(rem_pairs))
        per_core.append((lists, rem_pairs))

    lmax = max(((lmax + 127) // 128) * 128, 128)
    nrem = lmax // 128
    nsc = NSLOT * MCAP + nrem

    tables = []
    for lists, rem_pairs in per_core:
        scidx = np.full((128, nsc), SENTINEL, dtype=np.int32)
        for gg, dests in lists.items():
            q, p = divmod(gg, 128)
            for m, dd in enumerate(dests[:MCAP]):
                scidx[p, m * NSLOT + q] = 2 * dd + 1
        gtok = np.zeros(lmax, dtype=np.int64)
        for t, (gg, dd) in enumerate(rem_pairs):
            gtok[t] = gg
            scidx[t % 128, NSLOT * MCAP + t // 128] = 2 * dd + 1
        tables.append((scidx, _wrap16(gtok)))
    return tables, lmax


def kernel(curr, targets):
    from concourse.bass_utils import run_bass_kernel_spmd

    curr = np.ascontiguousarray(np.asarray(curr), dtype=np.float32)
    targets = np.asarray(targets).astype(np.int32)

    tables, lmax = _scatter_tables(targets)
    key = ("nc", lmax)
    if key not in _CACHE:
        _CACHE[key] = _build_nc(lmax)
    nc = _CACHE[key]
    _CACHE["last"] = nc
    _CACHE["last_lmax"] = lmax

    in_maps = [
        {"curr": curr, "scidx": tables[core][0], "gidx": tables[core][1]}
        for core in range(NCORES)
    ]
    res = None
    for attempt in range(3):
        try:
            res = run_bass_kernel_spmd(nc, in_maps, core_ids=list(range(NCORES)))
            break
        except Exception:
            if attempt == 2:
                raise
            import time

            time.sleep(5.0 * (attempt + 1))
    expanded = np.concatenate([r["out"] for r in res.results], axis=0)

    new_targets = np.tile(
        np.concatenate([np.ones(K, np.int32), np.zeros(C - 1, np.int32)]), N
    )
    return new_targets, expanded
